# revision 58
# baseline (speedup 1.0000x reference)
"""Cache-offloaded transformer decode step on 8 TRN2 NeuronCores.

Sharding: heads 2/core (attention fully local), FFN tensor-parallel
1024/core, vocab 4000/core; per-layer AllGather of head outputs and
AllGather+local-reduce of FFN partial sums; one final AllGather for the
softmax normalizer.

Device layout convention: activations live transposed — hidden dim on
partitions (16 chunks of 128), batch on free axis. x is the tile
XT[128, 32] with col = b*16 + c meaning x[b, c*128 + p].

Weights and KV are bf16 on device (f32 PSUM accumulation); the
residual stream, layernorm, softmax and collectives stay f32.
"""

import math
import os
import sys

import numpy as np

DBG = os.environ.get("KDBG", "0") == "1"

for _p in ("/opt/trn_rl_repo",):
    if _p not in sys.path:
        sys.path.append(_p)

import concourse.bass as bass
import concourse.bacc as bacc
import concourse.mybir as mybir
import concourse.tile as tile
from concourse.bass_utils import run_bass_kernel_spmd

F32 = mybir.dt.float32
BF16 = mybir.dt.bfloat16
AF = mybir.ActivationFunctionType
AX = mybir.AxisListType

NCORES = 8
B, S, HID, NH, L, V = 2, 2048, 2048, 16, 8, 32000
D = HID // NH                     # 128
OFF = (S + 1) // 2                # 1024
R = S - OFF + 1                   # 1025
NROWS = 7 * R                     # 7175 heap rows actually read
HPC = NH // NCORES                # heads per core = 2
PAIRS = B * HPC                   # (b, local head) pairs per core = 4
QKV_PC = 3 * D * HPC              # qkv_w rows per core = 768
FFN_PC = 4 * HID // NCORES        # ffn hidden per core = 1024
VPC = V // NCORES                 # vocab per core = 4000
VPCP = 4096                       # padded to 32*128
NC16 = HID // 128                 # 16 hid chunks
KCH = 17                          # key chunks per attention (2049 keys)
INVSQ = 1.0 / math.sqrt(D)

_cached = None


def _ln(nc, sb, ps, XT, gT, bT, ones, eps, name):
    """LayerNorm of XT -> (xn_f32, xn_bf16) tiles [128, 32]."""
    sq = sb.tile([128, 32], F32, tag="ln_sq", name=f"sq_{name}")
    nc.vector.tensor_mul(sq[:], XT[:], XT[:])
    # two matmuls into one PSUM bank: memset + start=False (a start=True
    # pair would clobber each other's region via the bank clear)
    st = ps.tile([1, 64], F32, tag="ps_small", name=f"st_{name}")
    nc.vector.memset(st[:], 0.0)
    nc.tensor.matmul(st[0:1, 0:32], ones[:], XT[:], start=False, stop=True)
    nc.tensor.matmul(st[0:1, 32:64], ones[:], sq[:], start=False, stop=True)
    red = sb.tile([1, 4], F32, tag="ln_red", name=f"red_{name}")
    nc.vector.reduce_sum(
        red[:], st[0:1, :].rearrange("p (t b c) -> p (t b) c", t=2, b=2), axis=AX.X
    )
    mr = sb.tile([1, 4], F32, tag="ln_mr", name=f"mr_{name}")
    # mr = [mean0, mean1, E[x^2]_0, E[x^2]_1]
    nc.vector.tensor_scalar_mul(mr[0:1, :], red[0:1, :], 1.0 / HID)
    var = sb.tile([1, 4], F32, tag="ln_var", name=f"var_{name}")
    nc.vector.tensor_mul(var[0:1, 0:2], mr[0:1, 0:2], mr[0:1, 0:2])
    nc.vector.tensor_sub(var[0:1, 0:2], mr[0:1, 2:4], var[0:1, 0:2])
    nc.scalar.activation(var[0:1, 2:4], var[0:1, 0:2], AF.Sqrt,
                         bias=eps[0:1, 0:1])
    nc.vector.reciprocal(mr[0:1, 2:4], var[0:1, 2:4])
    mrb = sb.tile([128, 4], F32, tag="ln_mrb", name=f"mrb_{name}")
    nc.gpsimd.partition_broadcast(mrb[:], mr[0:1, :])

    xn = sb.tile([128, 32], F32, tag="xn", name=f"xn_{name}")
    xv = xn[:, :].rearrange("p (b c) -> p b c", b=2)
    XTv = XT[:, :].rearrange("p (b c) -> p b c", b=2)
    m_b = mrb[:, 0:2].unsqueeze(2).broadcast_to([128, 2, 16])
    r_b = mrb[:, 2:4].unsqueeze(2).broadcast_to([128, 2, 16])
    g_b = gT[:, :].unsqueeze(1).broadcast_to([128, 2, 16])
    b_b = bT[:, :].unsqueeze(1).broadcast_to([128, 2, 16])
    nc.vector.tensor_sub(xv, XTv, m_b)
    nc.vector.tensor_mul(xv, xv, r_b)
    nc.vector.tensor_mul(xv, xv, g_b)
    xnb = sb.tile([128, 32], BF16, tag="xnb", name=f"xnb_{name}")
    xnbv = xnb[:, :].rearrange("p (b c) -> p b c", b=2)
    nc.vector.tensor_add(xnbv, xv, b_b)
    return xn, xnb


def _build():
    nc = bacc.Bacc("TRN2", target_bir_lowering=False, debug=False,
                   num_devices=NCORES)
    rg = [list(range(NCORES))]

    xpet = nc.dram_tensor("xpet", [128, 32], F32, kind="ExternalInput").ap()
    lng = nc.dram_tensor("lng_t", [128, 16], F32, kind="ExternalInput").ap()
    lnb = nc.dram_tensor("lnb_t", [128, 16], F32, kind="ExternalInput").ap()
    ident_in = nc.dram_tensor("ident", [128, 128], BF16, kind="ExternalInput").ap()
    qkv_w = nc.dram_tensor("qkv_wt", [L, HID, QKV_PC], BF16,
                           kind="ExternalInput").ap()
    f1_w = nc.dram_tensor("ffn1_wt", [L, HID, FFN_PC], BF16,
                          kind="ExternalInput").ap()
    f2_w = nc.dram_tensor("ffn2_wt", [L, FFN_PC, HID], BF16,
                          kind="ExternalInput").ap()
    ow = nc.dram_tensor("out_wt", [HID, VPCP], BF16, kind="ExternalInput").ap()
    kt_heap = nc.dram_tensor("kt_heap", [PAIRS, 128, NROWS], BF16,
                             kind="ExternalInput").ap()
    v_heap = nc.dram_tensor("v_heap_s", [PAIRS, NROWS, 128], BF16,
                            kind="ExternalInput").ap()
    kt_off = nc.dram_tensor("kt_off", [L - 1, PAIRS, 128, OFF], BF16,
                            kind="ExternalInput").ap()
    v_off = nc.dram_tensor("v_off_s", [L - 1, PAIRS, OFF, 128], BF16,
                           kind="ExternalInput").ap()
    out = nc.dram_tensor("out", [128, 64], F32, kind="ExternalOutput").ap()

    dbg_specs = {}
    if DBG:
        for l in range(L):
            dbg_specs[f"dbg_xt_attn{l}"] = [128, 32]
            dbg_specs[f"dbg_xt_ffn{l}"] = [128, 32]
        dbg_specs.update({
            "dbg_xn0": [128, 32], "dbg_o0": [128, 4],
            "dbg_xd0": [128, 32], "dbg_ffn0": [128, 32],
            "dbg_logT": [128, 64], "dbg_tot": [1, 2],
        })
    dbg_outs = {k: nc.dram_tensor(k, v, F32, kind="ExternalOutput").ap()
                for k, v in dbg_specs.items()}

    def dbg(name, ap):
        if DBG and name in dbg_outs:
            nc.scalar.dma_start(dbg_outs[name][:], ap)

    with tile.TileContext(nc) as tc:
        with (
            tc.tile_pool(name="sb", bufs=3) as sb,
            tc.tile_pool(name="wq_p", bufs=4) as wq_p,
            tc.tile_pool(name="f1_p", bufs=4) as f1_p,
            tc.tile_pool(name="f2_p", bufs=4) as f2_p,
            tc.tile_pool(name="ow_p", bufs=4) as ow_p,
            tc.tile_pool(name="kt_p", bufs=7) as kt_p,
            tc.tile_pool(name="vt_p", bufs=7) as vt_p,
            tc.tile_pool(name="ps", bufs=3, space="PSUM") as ps,
            tc.tile_pool(name="dram", bufs=2, space="DRAM") as dram,
        ):
            # ---- persistent small tiles ----
            XT = sb.tile([128, 32], F32, bufs=1, name="XT")
            gT = sb.tile([128, 16], F32, bufs=1, name="gT")
            bT = sb.tile([128, 16], F32, bufs=1, name="bT")
            ones = sb.tile([128, 1], F32, bufs=1, name="ones")
            ident = sb.tile([128, 128], BF16, bufs=1, name="ident")
            nc.scalar.dma_start(XT[:], xpet[:])
            nc.scalar.dma_start(gT[:], lng[:])
            nc.scalar.dma_start(bT[:], lnb[:])
            nc.scalar.dma_start(ident[:], ident_in[:])
            nc.vector.memset(ones[:], 1.0)
            eps = sb.tile([1, 1], F32, bufs=1, name="eps")
            nc.vector.memset(eps[:], 1e-5)
            knew = [sb.tile([128, 4], BF16, bufs=1, name=f"knew{l}")
                    for l in range(L - 2)]
            vnew = [sb.tile([4, 128], BF16, bufs=1, name=f"vnew{l}")
                    for l in range(L - 2)]

            for l in range(L):
                # ---------- LN1 + QKV ----------
                xn, xnb = _ln(nc, sb, ps, XT, gT, bT, ones, eps, f"l{l}a")
                if l == 0:
                    dbg("dbg_xn0", xn[:])
                psq = ps.tile([128, 12], F32, tag="ps_big", name=f"psq{l}")
                nc.vector.memset(psq[:], 0.0)
                for g in range(4):
                    wt = wq_p.tile([128, 4 * QKV_PC], BF16, tag="wq",
                                   name=f"wq{l}_{g}")
                    nc.sync.dma_start(
                        wt[:, :].rearrange("p (c f) -> p c f", c=4),
                        qkv_w[l, 512 * g:512 * (g + 1), :].rearrange(
                            "(c p) f -> p c f", c=4),
                    )
                    for cl in range(4):
                        rhs = xnb[:, (4 * g + cl)::16]
                        for j in range(6):
                            nc.tensor.matmul(
                                psq[:, 2 * j:2 * j + 2],
                                wt[:, QKV_PC * cl + 128 * j:
                                   QKV_PC * cl + 128 * (j + 1)],
                                rhs, start=False,
                                stop=(g == 3 and cl == 3),
                            )
                # extract qT, k_new, v_newT; psq col = 6*jh + 2*t + b
                qT = sb.tile([128, 4], BF16, tag="qT", name=f"qT{l}")
                for b in range(2):
                    # dst col pi = 2*b + jh ; src col = 6*jh + 2*t + b
                    nc.vector.tensor_copy(qT[:, 2 * b:2 * b + 2],
                                          psq[:, b:b + 7:6])
                if l < L - 2:
                    vnT = sb.tile([128, 4], BF16, tag="vnT", name=f"vnT{l}")
                    for b in range(2):
                        nc.vector.tensor_copy(knew[l][:, 2 * b:2 * b + 2],
                                              psq[:, 2 + b:2 + b + 7:6])
                        nc.vector.tensor_copy(vnT[:, 2 * b:2 * b + 2],
                                              psq[:, 4 + b:4 + b + 7:6])
                    # transpose v_newT [128,4] -> v_new rows [4,128]
                    ptr = ps.tile([4, 128], BF16, tag="ps_small",
                                  name=f"ptr{l}")
                    nc.tensor.transpose(ptr[:], vnT[:], ident[:])
                    nc.vector.tensor_copy(vnew[l][:], ptr[:])

                # ---------- attention ----------
                rs = (l - 1) * R
                pss = ps.tile([128, PAIRS * KCH], F32, tag="ps_big",
                              name=f"pss{l}")
                # zero the whole tile; score matmuls accumulate onto it.
                # chunk-16 cols keep 0 in partitions 1..127 -> exp()=1,
                # corrected by the -127/pair constant below.
                nc.vector.memset(pss[:], 0.0)
                kts, vts = [], []
                for pi in range(PAIRS):
                    kt = kt_p.tile([128, 2049], BF16, tag="kt",
                                   name=f"kt{l}_{pi}")
                    vt = vt_p.tile([128, KCH * 128], BF16, tag="vt",
                                   name=f"vt{l}_{pi}")
                    kts.append(kt)
                    vts.append(vt)
                    if l == 0:
                        nc.sync.dma_start(kt[:, 0:2048], kt_heap[pi, :, 0:2048])
                        nc.sync.dma_start(
                            vt[:, 0:2048].rearrange("p (ch f) -> p ch f", ch=16),
                            v_heap[pi, 0:2048, :].rearrange(
                                "(ch p) f -> p ch f", ch=16),
                        )
                    else:
                        nc.sync.dma_start(kt[:, 0:R], kt_heap[pi, :, rs:rs + R])
                        nc.sync.dma_start(kt[:, R:2049], kt_off[l - 1, pi, :, :])
                        nc.sync.dma_start(
                            vt[:, 0:1024].rearrange("p (ch f) -> p ch f", ch=8),
                            v_heap[pi, rs:rs + 1024, :].rearrange(
                                "(ch p) f -> p ch f", ch=8),
                        )
                        nc.sync.dma_start(vt[0:1, 1024:1152],
                                          v_heap[pi, rs + 1024:rs + R, :])
                        nc.sync.dma_start(vt[1:128, 1024:1152],
                                          v_off[l - 1, pi, 0:127, :])
                        nc.sync.dma_start(
                            vt[:, 1152:2048].rearrange("p (ch f) -> p ch f", ch=7),
                            v_off[l - 1, pi, 127:1023, :].rearrange(
                                "(ch p) f -> p ch f", ch=7),
                        )
                        nc.sync.dma_start(vt[0:1, 2048:2176],
                                          v_off[l - 1, pi, 1023:1024, :])
                    # inject this layer's / layer l-2's new KV rows
                    if l == 0:
                        nc.vector.tensor_copy(kt[:, 2048:2049],
                                              knew[0][:, pi:pi + 1])
                        nc.gpsimd.dma_start(vt[0:1, 2048:2176],
                                            vnew[0][pi:pi + 1, :])
                    elif l >= 2:
                        nc.vector.tensor_copy(kt[:, 1023:1024],
                                              knew[l - 2][:, pi:pi + 1])
                        nc.gpsimd.dma_start(vt[127:128, 896:1024],
                                            vnew[l - 2][pi:pi + 1, :])
                # scores
                for pi in range(PAIRS):
                    kt = kts[pi]
                    for c in range(16):
                        nc.tensor.matmul(
                            pss[:, KCH * pi + c:KCH * pi + c + 1],
                            kt[:, 128 * c:128 * (c + 1)], qT[:, pi:pi + 1],
                            start=False, stop=True,
                        )
                    nc.tensor.matmul(
                        pss[0:1, KCH * pi + 16:KCH * pi + 17],
                        kt[:, 2048:2049], qT[:, pi:pi + 1],
                        start=False, stop=True,
                    )
                # softmax (no max subtraction; logits are O(1))
                prob = sb.tile([128, PAIRS * KCH], F32, tag="prob",
                               name=f"prob{l}")
                nc.scalar.activation(prob[:], pss[:], AF.Exp, scale=INVSQ)
                ssum = sb.tile([1, 4], F32, tag="ssum", name=f"ssum{l}")
                pssum = ps.tile([1, PAIRS * KCH], F32, tag="ps_small",
                                name=f"pssum{l}")
                nc.tensor.matmul(pssum[:], ones[:], prob[:], start=True,
                                 stop=True)
                sumsb = sb.tile([1, PAIRS * KCH], F32, tag="sumsb",
                                name=f"sumsb{l}")
                nc.vector.tensor_copy(sumsb[:], pssum[:])
                nc.vector.reduce_sum(
                    ssum[:],
                    sumsb[0:1, :].rearrange("p (q c) -> p q c", q=PAIRS),
                    axis=AX.X,
                )
                # chunk-16 col partitions 1..127 hold exp(0)=1 each
                nc.vector.tensor_scalar_add(ssum[:], ssum[:], -127.0)
                inv = sb.tile([1, 4], F32, tag="inv", name=f"inv{l}")
                nc.vector.reciprocal(inv[:], ssum[:])
                invb = sb.tile([128, 4], F32, tag="invb", name=f"invb{l}")
                nc.gpsimd.partition_broadcast(invb[:], inv[0:1, :])
                prob_b = sb.tile([128, PAIRS * KCH], BF16, tag="prob_b",
                                 name=f"prob_b{l}")
                nc.vector.tensor_mul(
                    prob_b[:, :].rearrange("p (q c) -> p q c", q=PAIRS),
                    prob[:, :].rearrange("p (q c) -> p q c", q=PAIRS),
                    invb[:, :].unsqueeze(2).broadcast_to([128, PAIRS, KCH]),
                )
                # o = probs @ V
                pso = ps.tile([128, 4], F32, tag="ps_big", name=f"pso{l}")
                nc.vector.memset(pso[:], 0.0)
                for pi in range(PAIRS):
                    vt = vts[pi]
                    for c in range(16):
                        nc.tensor.matmul(
                            pso[:, pi:pi + 1],
                            vt[:, 128 * c:128 * (c + 1)],
                            prob_b[:, KCH * pi + c:KCH * pi + c + 1],
                            start=False, stop=False,
                        )
                    nc.tensor.matmul(
                        pso[:, pi:pi + 1],
                        vt[0:1, 2048:2176],
                        prob_b[0:1, KCH * pi + 16:KCH * pi + 17],
                        start=False, stop=True,
                    )
                o_sb = sb.tile([128, 4], F32, tag="o_sb", name=f"o{l}")
                nc.vector.tensor_copy(o_sb[:], pso[:])
                if l == 0:
                    dbg("dbg_o0", o_sb[:])

                # ---------- AllGather head outputs, residual add ----------
                ag_in = dram.tile([128, 4], F32, tag="ag_in", name=f"agi{l}")
                ag_out = dram.tile([NCORES * 128, 4], F32, tag="ag_out",
                                   addr_space="Shared", name=f"ago{l}")
                nc.gpsimd.dma_start(ag_in[:], o_sb[:])
                nc.gpsimd.collective_compute(
                    "AllGather", mybir.AluOpType.bypass,
                    ins=[ag_in[:]], outs=[ag_out[:]], replica_groups=rg,
                )
                # readback in contiguous (r, pi) column order — 16B runs —
                # and permute to XT's (b, hid) order inside the DVE add
                xd = sb.tile([128, 32], F32, tag="xd", name=f"xd{l}")
                nc.scalar.dma_start(
                    xd[:, :].rearrange("p (r q) -> p r q", r=NCORES),
                    ag_out[:, :].rearrange("(r p) q -> p r q", r=NCORES),
                )
                nc.vector.tensor_add(
                    XT[:, :].rearrange("p (b r j) -> p b r j", b=2, r=NCORES),
                    XT[:, :].rearrange("p (b r j) -> p b r j", b=2, r=NCORES),
                    xd[:, :].rearrange("p (r b j) -> p b r j", r=NCORES, b=2),
                )
                dbg(f"dbg_xt_attn{l}", XT[:])

                # ---------- LN2 + FFN ----------
                xn2, xnb2 = _ln(nc, sb, ps, XT, gT, bT, ones, eps, f"l{l}b")
                psh = ps.tile([128, 16], F32, tag="ps_big", name=f"psh{l}")
                nc.vector.memset(psh[:], 0.0)
                for g in range(4):
                    wt = f1_p.tile([128, 4 * FFN_PC], BF16, tag="f1",
                                   name=f"f1{l}_{g}")
                    nc.sync.dma_start(
                        wt[:, :].rearrange("p (c f) -> p c f", c=4),
                        f1_w[l, 512 * g:512 * (g + 1), :].rearrange(
                            "(c p) f -> p c f", c=4),
                    )
                    for cl in range(4):
                        rhs = xnb2[:, (4 * g + cl)::16]
                        for j in range(8):
                            nc.tensor.matmul(
                                psh[:, 2 * j:2 * j + 2],
                                wt[:, FFN_PC * cl + 128 * j:
                                   FFN_PC * cl + 128 * (j + 1)],
                                rhs, start=False,
                                stop=(g == 3 and cl == 3),
                            )
                hT = sb.tile([128, 16], BF16, tag="hT", name=f"hT{l}")
                nc.scalar.activation(hT[:], psh[:], AF.Gelu)
                psf = ps.tile([128, 32], F32, tag="ps_big", name=f"psf{l}")
                nc.vector.memset(psf[:], 0.0)
                for g2 in range(4):
                    wt = f2_p.tile([128, 2 * HID], BF16, tag="f2",
                                   name=f"f2{l}_{g2}")
                    nc.sync.dma_start(
                        wt[:, :].rearrange("p (c f) -> p c f", c=2),
                        f2_w[l, 256 * g2:256 * (g2 + 1), :].rearrange(
                            "(c p) f -> p c f", c=2),
                    )
                    for ckl in range(2):
                        ck = 2 * g2 + ckl
                        rhs = hT[:, 2 * ck:2 * ck + 2]
                        for m in range(16):
                            nc.tensor.matmul(
                                psf[:, 2 * m:2 * m + 2],
                                wt[:, HID * ckl + 128 * m:
                                   HID * ckl + 128 * (m + 1)],
                                rhs, start=False,
                                stop=(g2 == 3 and ckl == 1),
                            )
                ffn_sb = sb.tile([128, 32], F32, tag="ffn_sb", name=f"ffn{l}")
                nc.vector.tensor_copy(ffn_sb[:], psf[:])
                # ---------- AllGather ffn partials, local reduce, add ----
                fag_in = dram.tile([128, 32], F32, tag="fag_in",
                                   name=f"fagi{l}")
                fag_out = dram.tile([NCORES * 128, 32], F32, tag="fag_out",
                                    addr_space="Shared", name=f"fago{l}")
                nc.gpsimd.dma_start(fag_in[:], ffn_sb[:])
                nc.gpsimd.collective_compute(
                    "AllGather", mybir.AluOpType.bypass,
                    ins=[fag_in[:]], outs=[fag_out[:]], replica_groups=rg,
                )
                fsum = sb.tile([128, NCORES * 32], F32, tag="fsum",
                               name=f"fsum{l}")
                nc.scalar.dma_start(
                    fsum[:, :].rearrange("p (r f) -> p r f", r=NCORES),
                    fag_out[:, :].rearrange("(r p) f -> p r f", r=NCORES),
                )
                ard = sb.tile([128, 32], F32, tag="ard", name=f"ard{l}")
                nc.vector.reduce_sum(
                    ard[:],
                    fsum[:, :].rearrange("p (r f) -> p f r", r=NCORES),
                    axis=AX.X,
                )
                if l == 0:
                    dbg("dbg_ffn0", psf[:])
                # ard cols are (hid-chunk m, b) interleaved; XT cols are
                # (b, hid-chunk) — permute via the AP
                nc.vector.tensor_add(
                    XT[:, :].rearrange("p (b c) -> p b c", b=2),
                    XT[:, :].rearrange("p (b c) -> p b c", b=2),
                    ard[:, :].rearrange("p (m b) -> p b m", b=2),
                )
                dbg(f"dbg_xt_ffn{l}", XT[:])

            # ---------- final LN + vocab head + softmax ----------
            xn3, xnb3 = _ln(nc, sb, ps, XT, gT, bT, ones, eps, "fin")
            psl = ps.tile([128, 64], F32, tag="ps_big", name="psl")
            nc.vector.memset(psl[:], 0.0)
            for c in range(NC16):
                wt = ow_p.tile([128, VPCP], BF16, tag="ow", name=f"ow{c}")
                nc.sync.dma_start(wt[:], ow[128 * c:128 * (c + 1), :])
                rhs = xnb3[:, c::16]
                for m in range(32):
                    nc.tensor.matmul(
                        psl[:, 2 * m:2 * m + 2],
                        wt[:, 128 * m:128 * (m + 1)], rhs,
                        start=False, stop=(c == NC16 - 1),
                    )
            logT = sb.tile([128, 64], F32, bufs=1, name="logT")
            nc.vector.tensor_copy(logT[:], psl[:])
            dbg("dbg_logT", logT[:])
            # output the UNNORMALIZED exp; the host holds every core's
            # chunk, so the softmax denominator is computed there — saves
            # a whole collective round-trip.
            E = sb.tile([128, 64], F32, bufs=1, name="E")
            nc.scalar.activation(E[:], logT[:], AF.Exp)
            nc.sync.dma_start(out[:], E[:])

    nc.compile()
    return nc


def _get_nc():
    global _cached
    if _cached is None:
        _cached = _build()
    return _cached


def _pos_encoding(pos):
    half = np.arange(HID // 2, dtype=np.float32)
    div = np.exp((-math.log(10000.0) * (2.0 * half) / HID).astype(np.float32))
    ang = np.float32(pos) * div
    pe = np.zeros((HID,), dtype=np.float32)
    pe[0::2] = np.sin(ang)
    pe[1::2] = np.cos(ang)
    return pe


def kernel(x, qkv_w, ffn1_w, ffn2_w, out_w, ln_g, ln_b,
           k_heap, v_heap, k_off, v_off, current_pos):
    import ml_dtypes
    bf16 = ml_dtypes.bfloat16

    x = np.asarray(x, dtype=np.float32)
    qkv_w = np.asarray(qkv_w, dtype=np.float32)
    ffn1_w = np.asarray(ffn1_w, dtype=np.float32)
    ffn2_w = np.asarray(ffn2_w, dtype=np.float32)
    out_w = np.asarray(out_w, dtype=np.float32)
    ln_g = np.asarray(ln_g, dtype=np.float32)
    ln_b = np.asarray(ln_b, dtype=np.float32)
    k_heap = np.asarray(k_heap, dtype=np.float32)
    v_heap = np.asarray(v_heap, dtype=np.float32)
    k_off = np.asarray(k_off, dtype=np.float32)
    v_off = np.asarray(v_off, dtype=np.float32)
    pos = int(np.asarray(current_pos))

    xpe = x.reshape(B, HID) + _pos_encoding(pos)[None, :]
    # XT[p, b*16+c] = x[b, c*128+p]
    xpet = np.ascontiguousarray(
        xpe.reshape(B, NC16, 128).transpose(2, 0, 1).reshape(128, B * NC16))
    lng_t = np.ascontiguousarray(ln_g.reshape(NC16, 128).T)
    lnb_t = np.ascontiguousarray(ln_b.reshape(NC16, 128).T)
    ident = np.eye(128, dtype=bf16)

    in_maps = []
    for c in range(NCORES):
        qs = qkv_w[:, QKV_PC * c:QKV_PC * (c + 1), :].transpose(0, 2, 1)
        f1 = ffn1_w[:, FFN_PC * c:FFN_PC * (c + 1), :].transpose(0, 2, 1)
        f2 = ffn2_w[:, :, FFN_PC * c:FFN_PC * (c + 1)].transpose(0, 2, 1)
        owt = np.zeros((HID, VPCP), dtype=bf16)
        owt[:, :VPC] = out_w[VPC * c:VPC * (c + 1), :].T.astype(bf16)
        h0, h1 = HPC * c, HPC * (c + 1)
        # pair pi = 2*b + jh
        kh = k_heap[:, h0:h1, :NROWS, :].reshape(B * HPC, NROWS, 128)
        kh_t = np.ascontiguousarray(kh.transpose(0, 2, 1).astype(bf16))
        vh_s = np.ascontiguousarray(
            v_heap[:, h0:h1, :NROWS, :].reshape(B * HPC, NROWS, 128)
            .astype(bf16))
        kt_o = np.ascontiguousarray(
            k_off[:, :, h0:h1, :, :].reshape(L - 1, B * HPC, OFF, 128)
            .transpose(0, 1, 3, 2).astype(bf16))
        vt_o = np.ascontiguousarray(
            v_off[:, :, h0:h1, :, :].reshape(L - 1, B * HPC, OFF, 128)
            .astype(bf16))
        in_maps.append({
            "xpet": xpet, "lng_t": lng_t, "lnb_t": lnb_t, "ident": ident,
            "qkv_wt": np.ascontiguousarray(qs.astype(bf16)),
            "ffn1_wt": np.ascontiguousarray(f1.astype(bf16)),
            "ffn2_wt": np.ascontiguousarray(f2.astype(bf16)),
            "out_wt": owt,
            "kt_heap": kh_t, "v_heap_s": vh_s,
            "kt_off": kt_o, "v_off_s": vt_o,
        })

    nc = _get_nc()
    try:
        res = run_bass_kernel_spmd(nc, in_maps, core_ids=list(range(NCORES)))
    except ModuleNotFoundError:
        # BASS_TRACE set but the axon NTFF hook module is absent in this
        # image — run untraced instead of failing.
        os.environ["BASS_NEVER_TRACE"] = "1"
        res = run_bass_kernel_spmd(nc, in_maps, core_ids=list(range(NCORES)))
    global LAST_RESULT
    LAST_RESULT = res

    # device returns unnormalized exp(logits); normalize on host
    expv = np.zeros((B, V), dtype=np.float32)
    for c in range(NCORES):
        o = res.results[c]["out"].reshape(128, 32, 2)
        for b in range(B):
            expv[b, VPC * c:VPC * (c + 1)] = \
                o[:, :, b].T.reshape(VPCP)[:VPC]
    probs = expv / expv.sum(axis=1, keepdims=True)
    return probs.reshape(B, 1, V).astype(np.float32)
